# revision 15
# baseline (speedup 1.0000x reference)
"""Trainium2 Bass kernel for nn_MultiHeadCrossAttention.

Sharding: 8 cores = 4 batches x 2 head-groups (8 heads each).

v2 pipeline: the attention computation is one global "score chunk"
stream.  A chunk is a [128, 512] fp32 score tile (kl-tile on
partitions, 512 q columns, one head).  Chunks rotate through a 6-bank
PSUM ring; the scalar engine exps them three at a time with N=1536
ACTIVATEs (amortizing the ~352-cycle per-instruction overhead and
keeping ACT ~100% busy).  attn@V accumulation and all projections
(k/q/v/out) time-share the remaining 2 PSUM banks, interleaved into
the chunk stream via a gated FIFO so the PE never idles long and the
first exp lands ~20us in.  Units are ordered c2-major so the
out-projection of the first q-half overlaps the second half's
attention.  Softmax denominators ride along as a 65th ones-column in
the attn@V matmuls; normalization multiplies by a DMA-broadcast
reciprocal row straight out of PSUM.  Host sums the two head-group
partials per batch and adds the output bias.
"""

import sys

sys.path.insert(0, "/opt/trn_rl_repo")

from collections import deque
from contextlib import ExitStack

import numpy as np
import ml_dtypes

import concourse.bass as bass
import concourse.bacc as bacc
import concourse.mybir as mybir
from concourse.tile import TileContext

DIM = 1024
H = 16
HD = 64
ROT = 32
B = 4
QL = 2048
KL = 2048
G = 2                # head-group (tensor-parallel) factor
HL = H // G          # 8 local heads
DL = HL * HD         # 512 local feature dims
NPAIR = HL // 2      # 4 head pairs
NCORE = 8

NMT = 16
NUNIT = 8
EXP_BUFS = 21
GATE = 6             # emit a chunk consumer once gs >= chunk + GATE

F32 = mybir.dt.float32
BF16 = mybir.dt.bfloat16
AFT = mybir.ActivationFunctionType
ALU = mybir.AluOpType
bf16 = ml_dtypes.bfloat16

# unit u -> (pair p, q-half c2); c2-major so outproj(qt 0-7) can start
# after unit 3.
UNITS = [(0, 0), (1, 0), (2, 0), (3, 0), (0, 1), (1, 1), (2, 1), (3, 1)]

_NC_CACHE = {}


def _rot_patterns():
    inv_freq = 1.0 / (10000.0 ** (np.arange(0, ROT, 2, dtype=np.float64) / ROT))
    t = np.arange(QL, dtype=np.float64)
    freqs = t[:, None] * inv_freq[None, :]          # [QL, 16]
    cos_p = np.ones((HD, QL), np.float64)
    sin_p = np.zeros((HD, QL), np.float64)
    for d in range(ROT):
        j = d // 2
        cos_p[d] = np.cos(freqs[:, j])
        sin_p[d] = np.sin(freqs[:, j]) * (-1.0 if d % 2 == 0 else 1.0)
    cc = np.tile(cos_p, (2, 1)).astype(np.float32)  # [128, QL]
    ss = np.tile(sin_p, (2, 1)).astype(np.float32)
    return cc, ss


def _build_nc():
    if "nc" in _NC_CACHE:
        return _NC_CACHE["nc"]
    nc = bacc.Bacc("TRN2", target_bir_lowering=False)

    d = {}
    for name, shape, dt in [
        ("qT", [DIM, QL], BF16), ("kT", [DIM, KL], BF16), ("vT", [DIM, KL], BF16),
        ("wqT", [DIM, DL], BF16), ("wkT", [DIM, DL], BF16), ("wvT", [DIM, DL], BF16),
        ("woT", [DL, DIM], BF16),
        ("bqp", [128, NPAIR], F32), ("bkp", [128, NPAIR], F32),
        ("bv", [1, DL], BF16), ("ones1", [1, 128], BF16),
        ("cc", [128, QL], BF16), ("ss", [128, QL], BF16),
    ]:
        d[name] = nc.dram_tensor(name, shape, dt, kind="ExternalInput")
    out_d = nc.dram_tensor("out", [QL, DIM], F32, kind="ExternalOutput")

    qT_t = d["qT"].rearrange("(a p) n -> a p n", p=128)     # [8, 128, QL]
    kT_t = d["kT"].rearrange("(a p) n -> a p n", p=128)
    vT_t = d["vT"].rearrange("(a p) n -> a p n", p=128)
    wqT_t = d["wqT"].rearrange("(a p) n -> a p n", p=128)   # [8, 128, DL]
    wkT_t = d["wkT"].rearrange("(a p) n -> a p n", p=128)
    wvT_t = d["wvT"].rearrange("(a p) n -> a p n", p=128)
    woT_t = d["woT"].rearrange("(a p) n -> a p n", p=128)   # [4, 128, DIM]
    out_t = out_d.rearrange("(a p) n -> a p n", p=128)      # [16, 128, DIM]

    SWAP_MASK = [(j + 1 if j % 2 == 0 else j - 1) for j in range(32)]

    with TileContext(nc) as tc, ExitStack() as top:
        # ---------------- persistent pools ----------------
        consts = top.enter_context(tc.tile_pool(name="consts", bufs=1))
        bq_s = consts.tile([128, NPAIR], F32)
        nc.gpsimd.dma_start(out=bq_s, in_=d["bqp"][:, :])
        bk_s = consts.tile([128, NPAIR], F32)
        nc.gpsimd.dma_start(out=bk_s, in_=d["bkp"][:, :])
        bv_s = consts.tile([1, DL], BF16)
        nc.gpsimd.dma_start(out=bv_s, in_=d["bv"][:, :])
        ones_s = consts.tile([1, 128], BF16)
        nc.gpsimd.dma_start(out=ones_s, in_=d["ones1"][:, :])
        cc_s = consts.tile([128, QL], BF16)
        nc.gpsimd.dma_start(out=cc_s, in_=d["cc"][:, :])
        ss_s = consts.tile([128, QL], BF16)
        nc.gpsimd.dma_start(out=ss_s, in_=d["ss"][:, :])
        warm = consts.tile([1, 8], F32)
        nc.scalar.activation(out=warm, in_=ones_s[0:1, 0:8], func=AFT.Exp)

        wq_pool = top.enter_context(tc.tile_pool(name="wq", bufs=1))
        wqs = [wq_pool.tile([128, DL], BF16, tag=f"wq{a}", name=f"wq{a}")
               for a in range(8)]
        kh_pool = top.enter_context(tc.tile_pool(name="kh", bufs=NPAIR))
        khT = [kh_pool.tile([128, KL], BF16, tag="kh", name=f"kh{p}")
               for p in range(NPAIR)]
        qh_pool = top.enter_context(tc.tile_pool(name="qh", bufs=2))
        vh_pool = top.enter_context(tc.tile_pool(name="vh", bufs=16))
        vh = [vh_pool.tile([128, NPAIR * 130], BF16, tag="vh", name=f"vh{t}")
              for t in range(16)]
        at_pool = top.enter_context(tc.tile_pool(name="atn", bufs=NPAIR))
        apT = [at_pool.tile([128, QL], BF16, tag="at", name=f"apT{p}")
               for p in range(NPAIR)]
        # q staging: two quarters [8][128,512] alive at once
        qst_pool = top.enter_context(tc.tile_pool(name="qst", bufs=1))
        qst = [[qst_pool.tile([128, 512], BF16, tag=f"qst{j}_{a}",
                              name=f"qs{j}_{a}")
                for a in range(8)] for j in range(2)]
        exp_pool = top.enter_context(tc.tile_pool(name="expp", bufs=EXP_BUFS))
        rot_pool = top.enter_context(tc.tile_pool(name="rot", bufs=1))
        rcp_pool = top.enter_context(tc.tile_pool(name="rcp", bufs=2))
        bt_pool = top.enter_context(tc.tile_pool(name="bt", bufs=1))
        dscr = top.enter_context(tc.tile_pool(name="dscr", bufs=4, space="DRAM"))

        ring_pool = top.enter_context(
            tc.tile_pool(name="ring", bufs=1, space="PSUM"))
        ring = ring_pool.tile([128, 6 * 512], F32)
        ab_pool = top.enter_context(
            tc.tile_pool(name="ab", bufs=2, space="PSUM"))

        qh_unit = [None] * NUNIT

        # ---------------- emission state ----------------
        st = {"gs": 0}
        exp_tiles = {}
        pending = deque()     # (gate_gs, closure) strict FIFO with gates

        def pump(max_ops=4):
            n = 0
            while pending and n < max_ops and pending[0][0] <= st["gs"]:
                pending.popleft()[1]()
                n += 1

        def drain(gate_limit=10**9):
            while pending and pending[0][0] <= gate_limit:
                pending.popleft()[1]()

        def emit_score_chunk(u, mt, h, n):
            p, c2 = UNITS[u]
            gs = st["gs"]
            slot = gs % 6
            nc.tensor.matmul(
                ring[:, slot * 512:(slot + 1) * 512],
                lhsT=khT[p][h * 64:(h + 1) * 64, mt * 128:(mt + 1) * 128],
                rhs=qh_unit[u][h * 64:(h + 1) * 64, n * 512:(n + 1) * 512],
                start=True, stop=True,
                tile_position=(h * 64, 0),
            )
            if gs % 3 == 2:
                i = gs // 3
                et = exp_pool.tile([128, 1536], BF16, tag="exp")
                base = (slot - 2) * 512
                nc.scalar.activation(out=et, in_=ring[:, base:base + 1536],
                                     func=AFT.Exp, scale=0.125)
                exp_tiles[i] = et
            st["gs"] = gs + 1

        def exp_slice(c):
            i, off = c // 3, (c % 3) * 512
            return exp_tiles[i][:, off:off + 512]

        # ---------------- op builders ----------------
        def add_attn_group(u, h, n, gate_fn):
            """16 accumulating attn@V MMs + normalize for group (u,h,n)."""
            p, c2 = UNITS[u]
            base_c = u * 64
            pa = ab_pool.tile([128, 512], F32, tag="ab", name=f"pa{u}{h}{n}")

            def mk(t):
                def op():
                    nc.tensor.matmul(
                        pa[0:65, :],
                        lhsT=vh[t][:, p * 130 + h * 65: p * 130 + (h + 1) * 65],
                        rhs=exp_slice(base_c + t * 4 + h * 2 + n),
                        start=(t == 0), stop=(t == 15),
                    )
                return op
            last_gate = 0
            for t in range(16):
                last_gate = gate_fn(t)
                pending.append((last_gate, mk(t)))

            def norm():
                rcp = rcp_pool.tile([1, 512], F32, tag="rcp")
                nc.vector.reciprocal(out=rcp, in_=pa[64:65, :])
                ds = dscr.tile([1, 512], F32, tag="ds")
                nc.sync.dma_start(out=ds, in_=rcp)
                bt = bt_pool.tile([64, 512], F32, tag="bt")
                nc.sync.dma_start(out=bt, in_=ds[0:1, :].to_broadcast([64, 512]))
                qbase = c2 * 1024 + n * 512
                nc.vector.tensor_tensor(
                    out=apT[p][h * 64:(h + 1) * 64, qbase:qbase + 512],
                    in0=pa[0:64, :], in1=bt, op=ALU.mult)
            pending.append((last_gate, norm))

        def add_attn_half(u, n, base_gate, step=1):
            """Both heads' accumulation groups for q-half n of unit u."""
            for h in range(2):
                add_attn_group(
                    u, h, n,
                    lambda t, h=h: max(u * 64 + t * 4 + h * 2 + n + GATE,
                                       base_gate + t * step))

        def add_proj_chunk(ws, xs, b_s, dst, pcol, dslice, xslice, gate=0):
            """dst[:, dslice] = sum_a ws[a][:, pcol*128:+128].T @ xs[a] + b."""
            ps = ab_pool.tile([128, 512], F32, tag="ab")

            def mk(a):
                def op():
                    nc.tensor.matmul(
                        ps,
                        lhsT=ws[a][:, pcol * 128:(pcol + 1) * 128],
                        rhs=xs[a][:, xslice] if xslice is not None else xs[a][:, :],
                        start=(a == 0), stop=(a == 7),
                    )
                return op
            for a in range(8):
                pending.append((gate, mk(a)))

            def bias():
                nc.vector.tensor_scalar_add(
                    out=dst[:, dslice], in0=ps,
                    scalar1=b_s[:, pcol:pcol + 1])
            pending.append((gate, bias))

        def add_rotary(dst, dst_base, cs_base, gate=0):
            """Partial rotary on dst[:, dst_base:dst_base+512] in place."""
            def op():
                dcol = slice(dst_base, dst_base + 512)
                ccol = slice(cs_base, cs_base + 512)
                sw = rot_pool.tile([128, 512], BF16, tag="sw")
                nc.vector.stream_shuffle(out=sw, in_=dst[:, dcol],
                                         mask=SWAP_MASK)
                t1 = rot_pool.tile([128, 512], BF16, tag="t1")
                nc.vector.tensor_tensor(out=t1, in0=sw, in1=ss_s[:, ccol],
                                        op=ALU.mult)
                t2 = rot_pool.tile([128, 512], BF16, tag="t2")
                nc.vector.tensor_tensor(out=t2, in0=dst[:, dcol],
                                        in1=cc_s[:, ccol], op=ALU.mult)
                nc.vector.tensor_tensor(out=dst[:, dcol], in0=t1, in1=t2,
                                        op=ALU.add)
            pending.append((gate, op))

        def add_qproj(u, gate=0):
            p, c2 = UNITS[u]
            qh = qh_pool.tile([128, 1024], BF16, tag="qh", name=f"qh{u}")
            qh_unit[u] = qh
            for n in range(2):
                add_proj_chunk(wqs, qst[n], bq_s, qh, p,
                               slice(n * 512, (n + 1) * 512), None, gate=gate)
                add_rotary(qh, n * 512, c2 * 1024 + n * 512, gate=gate)

        # ---------------- scoped staging pools ----------------
        scA = ExitStack()
        wk_pool = scA.enter_context(tc.tile_pool(name="wk", bufs=1))
        wks = [wk_pool.tile([128, DL], BF16, tag=f"wk{a}", name=f"wk{a}")
               for a in range(8)]
        wv_pool = scA.enter_context(tc.tile_pool(name="wv", bufs=1))
        wvs = [wv_pool.tile([128, DL], BF16, tag=f"wv{a}", name=f"wv{a}")
               for a in range(8)]
        kv_pool = scA.enter_context(tc.tile_pool(name="kv", bufs=1))
        # 4 rotating quarter-sets: hold kT cols [s*512:(s+1)*512] for all 8
        # a-tiles, overwritten with the same vT quarter once kproj is done.
        kvq = [[kv_pool.tile([128, 512], BF16, tag=f"kvq{s}{a}",
                             name=f"kvq{s}{a}") for a in range(8)]
               for s in range(4)]

        def add_kproj_q(p, s, gate=0):
            """khT[p] columns of quarter s + rotary."""
            col = slice(s * 512, (s + 1) * 512)
            add_proj_chunk(wks, kvq[s], bk_s, khT[p], p, col, None, gate=gate)
            add_rotary(khT[p], s * 512, s * 512, gate=gate)

        def add_vdma(s, gate=0):
            def op():
                for a in range(8):
                    eng = nc.sync if a % 2 == 0 else nc.gpsimd
                    eng.dma_start(out=kvq[s][a],
                                  in_=vT_t[a][:, s * 512:(s + 1) * 512])
            pending.append((gate, op))

        def add_vproj(t, gate=0):
            s, r = divmod(t, 4)
            ps = ab_pool.tile([128, DL], F32, tag="ab")

            def mk(a):
                def op():
                    nc.tensor.matmul(
                        ps,
                        lhsT=kvq[s][a][:, r * 128:(r + 1) * 128],
                        rhs=wvs[a],
                        start=(a == 0), stop=False,
                    )
                return op
            for a in range(8):
                pending.append((gate, mk(a)))

            def tail():
                nc.tensor.matmul(ps, lhsT=ones_s, rhs=bv_s,
                                 start=False, stop=True)
                vtr = vh[t].rearrange("p (g h e) -> p g h e", h=2, e=65)
                nc.vector.memset(vtr[:, :, :, 64:65], 1.0)
                psr = ps.rearrange("p (g h e) -> p g h e", h=2, e=64)
                nc.vector.tensor_copy(out=vtr[:, :, :, 0:64], in_=psr)
            pending.append((gate, tail))

        # ---------------- staging DMAs ----------------
        for s in range(4):
            for a in range(8):
                eng = nc.sync if a % 2 == 0 else nc.gpsimd
                eng.dma_start(out=kvq[s][a],
                              in_=kT_t[a][:, s * 512:(s + 1) * 512])
        for a in range(8):
            nc.gpsimd.dma_start(out=wks[a], in_=wkT_t[a])
        for a in range(8):
            nc.gpsimd.dma_start(out=wqs[a], in_=wqT_t[a])
        for a in range(8):
            nc.sync.dma_start(out=qst[0][a], in_=qT_t[a][:, 0:512])
            nc.sync.dma_start(out=qst[1][a], in_=qT_t[a][:, 512:1024])
        for a in range(8):
            nc.gpsimd.dma_start(out=wvs[a], in_=wvT_t[a])

        # ---------------- head: kproj(p0,q0) + qproj(u0) ----------------
        add_kproj_q(0, 0)
        add_qproj(0)
        drain()

        wo_holder = {}
        ot_tiles = {}

        def add_outproj(qt, dc, pairs, add_to, gate):
            ps = ab_pool.tile([128, 512], F32, tag="ab")

            def mk(j, p):
                def op():
                    nc.tensor.matmul(
                        ps,
                        lhsT=apT[p][:, qt * 128:(qt + 1) * 128],
                        rhs=wo_holder["wo"][p][:, dc * 512:(dc + 1) * 512],
                        start=(j == 0), stop=(j == len(pairs) - 1),
                    )
                return op
            for j, p in enumerate(pairs):
                pending.append((gate, mk(j, p)))

            def out():
                ot = ot_tiles[qt]
                sl = slice(dc * 512, (dc + 1) * 512)
                if add_to:
                    nc.vector.tensor_tensor(out=ot[:, sl], in0=ps,
                                            in1=ot[:, sl], op=ALU.add)
                else:
                    nc.vector.tensor_copy(out=ot[:, sl], in_=ps)
            pending.append((gate, out))

        def add_out_dma(qt, gate):
            def op():
                nc.sync.dma_start(out=out_t[qt], in_=ot_tiles[qt])
            pending.append((gate, op))

        # ---------------- unit windows ----------------
        def add_attn_unit(u):
            w = u * 64
            # n0: lag-consumed through window u; n1: early next window
            add_attn_half(u, 0, w + 10, step=2)
            add_attn_half(u, 1, w + 66)

        for u in range(NUNIT):
            w = u * 64
            if u == 0:
                # quarter pipeline: finish kproj per quarter, swap in vT,
                # vproj, then unit-0 attnV; all inside window 0.
                for s in range(4):
                    for p in range(4):
                        if (p, s) != (0, 0):
                            add_kproj_q(p, s, gate=s * 6 + 2)
                    add_vdma(s, gate=s * 6 + 3)
                for t in range(16):
                    add_vproj(t, gate=8 + 2 * t)
                add_qproj(1, gate=34)
                add_attn_unit(0)
            else:
                if u < 7:
                    add_qproj(u + 1, gate=w + 2)
                if u == 2:
                    def reload_q():
                        for a in range(8):
                            nc.sync.dma_start(out=qst[0][a],
                                              in_=qT_t[a][:, 1024:1536])
                            nc.sync.dma_start(out=qst[1][a],
                                              in_=qT_t[a][:, 1536:2048])
                    pending.append((w + 12, reload_q))
                qt_map = {4: [0, 1, 2], 5: [3, 4, 5], 6: [6, 7]}
                if u in qt_map:
                    for qt in qt_map[u]:
                        for dc in range(2):
                            add_outproj(qt, dc, list(range(NPAIR)), False,
                                        gate=w + 6)
                        add_out_dma(qt, gate=w + 30)
                add_attn_unit(u)

            for mt in range(NMT):
                for h in range(2):
                    for n in range(2):
                        pump(6)
                        emit_score_chunk(u, mt, h, n)

            if u == 0:
                # staging consumers (kproj/vproj/vdma) all have gates
                # <= ~40; pop them so the staging pools can be released.
                drain(w + 40)
                scA.close()
                wo_pool = top.enter_context(tc.tile_pool(name="wo", bufs=1))
                wo_s = [wo_pool.tile([128, DIM], BF16, tag=f"wo{p}",
                                     name=f"wo{p}") for p in range(NPAIR)]
                for p in range(NPAIR):
                    nc.gpsimd.dma_start(out=wo_s[p], in_=woT_t[p])
                wo_holder["wo"] = wo_s
                ot_pool = top.enter_context(tc.tile_pool(name="ot", bufs=2))
                for qt in range(16):
                    ot_tiles[qt] = ot_pool.tile([128, DIM], F32, tag="ot",
                                                name=f"ot{qt}")

        # ---------------- tail ----------------
        # leftover exp: 512 chunks = 170 full N=1536 instrs + 2 chunks.
        # Dedicated tile (not the rotating pool) so its slot has no WAR
        # against unit 7's still-pending attn consumers.
        et = consts.tile([128, 1024], BF16)
        nc.scalar.activation(out=et, in_=ring[:, 0:1024],
                             func=AFT.Exp, scale=0.125)
        exp_tiles[170] = et
        drain()
        for qt in range(8, 16):
            for dc in range(2):
                add_outproj(qt, dc, list(range(NPAIR)), False, gate=0)
            add_out_dma(qt, gate=0)
        drain()

    nc.compile()
    _NC_CACHE["nc"] = nc
    return nc


def _make_in_maps(q, k, v, Wq, bq, Wk, bk, Wv, bv, Wo, bo):
    q, k, v = (np.asarray(x, np.float32) for x in (q, k, v))
    Wq, Wk, Wv, Wo = (np.asarray(x, np.float32) for x in (Wq, Wk, Wv, Wo))
    bq, bk, bv, bo = (np.asarray(x, np.float32) for x in (bq, bk, bv, bo))
    cc, ss = _rot_patterns()
    ones1 = np.ones((1, 128), np.float32)
    in_maps = []
    for c in range(NCORE):
        b, g = divmod(c, G)
        gs = slice(g * DL, (g + 1) * DL)
        in_maps.append({
            "qT": np.ascontiguousarray(q[b].T).astype(bf16),
            "kT": np.ascontiguousarray(k[b].T).astype(bf16),
            "vT": np.ascontiguousarray(v[b].T).astype(bf16),
            "wqT": np.ascontiguousarray(Wq[gs, :].T).astype(bf16),
            "wkT": np.ascontiguousarray(Wk[gs, :].T).astype(bf16),
            "wvT": np.ascontiguousarray(Wv[gs, :].T).astype(bf16),
            "woT": np.ascontiguousarray(Wo[:, gs].T).astype(bf16),
            "bqp": np.ascontiguousarray(bq[gs].reshape(NPAIR, 128).T),
            "bkp": np.ascontiguousarray(bk[gs].reshape(NPAIR, 128).T),
            "bv": np.ascontiguousarray(bv[gs][None, :]).astype(bf16),
            "ones1": ones1.astype(bf16),
            "cc": cc.astype(bf16), "ss": ss.astype(bf16),
        })
    return in_maps


def run(inputs: dict, trace: bool = False, tmpdir: str | None = None):
    """Returns (out [B, QL, DIM] f32, exec_time_ns or None)."""
    from concourse.bass_utils import run_bass_kernel_spmd

    nc = _build_nc()
    in_maps = _make_in_maps(**inputs)
    res = run_bass_kernel_spmd(nc, in_maps, list(range(NCORE)), trace=trace,
                               tmpdir=tmpdir)
    bo = np.asarray(inputs["bo"], np.float32)
    outs = [res.results[i]["out"] for i in range(NCORE)]
    out = np.stack([outs[G * b] + outs[G * b + 1] for b in range(B)])
    out += bo[None, None, :]
    return out.astype(np.float32), res.exec_time_ns


def kernel(**inputs) -> np.ndarray:
    out, _ = run(inputs, trace=False)
    return out


# revision 18
# speedup vs baseline: 1.1299x; 1.1299x over previous
"""Trainium2 Bass kernel for nn_MultiHeadCrossAttention.

Sharding: 8 cores = 4 batches x 2 head-groups (8 heads each).

v2 pipeline: the attention computation is one global "score chunk"
stream.  A chunk is a [128, 512] fp32 score tile (kl-tile on
partitions, 512 q columns, one head).  Chunks rotate through a 6-bank
PSUM ring; the scalar engine exps them three at a time with N=1536
ACTIVATEs (amortizing the ~352-cycle per-instruction overhead and
keeping ACT ~100% busy).  attn@V accumulation and all projections
(k/q/v/out) time-share the remaining 2 PSUM banks, interleaved into
the chunk stream via a gated FIFO so the PE never idles long and the
first exp lands ~20us in.  Units are ordered c2-major so the
out-projection of the first q-half overlaps the second half's
attention.  Softmax denominators ride along as a 65th ones-column in
the attn@V matmuls; normalization multiplies by a DMA-broadcast
reciprocal row straight out of PSUM.  Host sums the two head-group
partials per batch and adds the output bias.
"""

import sys

sys.path.insert(0, "/opt/trn_rl_repo")

from collections import deque
from contextlib import ExitStack

import numpy as np
import ml_dtypes

import concourse.bass as bass
import concourse.bacc as bacc
import concourse.mybir as mybir
from concourse.tile import TileContext

DIM = 1024
H = 16
HD = 64
ROT = 32
B = 4
QL = 2048
KL = 2048
G = 2                # head-group (tensor-parallel) factor
HL = H // G          # 8 local heads
DL = HL * HD         # 512 local feature dims
NPAIR = HL // 2      # 4 head pairs
NCORE = 8

NMT = 16
NUNIT = 8
EXP_BUFS = 21
GATE = 8             # emit a chunk consumer once gs >= chunk + GATE

F32 = mybir.dt.float32
BF16 = mybir.dt.bfloat16
AFT = mybir.ActivationFunctionType
ALU = mybir.AluOpType
bf16 = ml_dtypes.bfloat16

# unit u -> (pair p, q-half c2); c2-major so outproj(qt 0-7) can start
# after unit 3.
UNITS = [(0, 0), (1, 0), (2, 0), (3, 0), (0, 1), (1, 1), (2, 1), (3, 1)]

_NC_CACHE = {}


def _rot_patterns():
    inv_freq = 1.0 / (10000.0 ** (np.arange(0, ROT, 2, dtype=np.float64) / ROT))
    t = np.arange(QL, dtype=np.float64)
    freqs = t[:, None] * inv_freq[None, :]          # [QL, 16]
    cos_p = np.ones((HD, QL), np.float64)
    sin_p = np.zeros((HD, QL), np.float64)
    for d in range(ROT):
        j = d // 2
        cos_p[d] = np.cos(freqs[:, j])
        sin_p[d] = np.sin(freqs[:, j]) * (-1.0 if d % 2 == 0 else 1.0)
    cc = np.tile(cos_p, (2, 1)).astype(np.float32)  # [128, QL]
    ss = np.tile(sin_p, (2, 1)).astype(np.float32)
    return cc, ss


def _build_nc():
    if "nc" in _NC_CACHE:
        return _NC_CACHE["nc"]
    nc = bacc.Bacc("TRN2", target_bir_lowering=False)

    d = {}
    for name, shape, dt in [
        ("qT", [DIM, QL], BF16), ("kT", [DIM, KL], BF16), ("vT", [DIM, KL], BF16),
        ("wqT", [DIM, DL], BF16), ("wkT", [DIM, DL], BF16), ("wvT", [DIM, DL], BF16),
        ("woT", [DL, DIM], BF16),
        ("bqp", [128, NPAIR], F32), ("bkp", [128, NPAIR], F32),
        ("bv", [1, DL], BF16), ("ones1", [1, 128], BF16),
        ("cc", [128, QL], BF16), ("ss", [128, QL], BF16),
    ]:
        d[name] = nc.dram_tensor(name, shape, dt, kind="ExternalInput")
    out_d = nc.dram_tensor("out", [QL, DIM], F32, kind="ExternalOutput")

    qT_t = d["qT"].rearrange("(a p) n -> a p n", p=128)     # [8, 128, QL]
    kT_t = d["kT"].rearrange("(a p) n -> a p n", p=128)
    vT_t = d["vT"].rearrange("(a p) n -> a p n", p=128)
    wqT_t = d["wqT"].rearrange("(a p) n -> a p n", p=128)   # [8, 128, DL]
    wkT_t = d["wkT"].rearrange("(a p) n -> a p n", p=128)
    wvT_t = d["wvT"].rearrange("(a p) n -> a p n", p=128)
    woT_t = d["woT"].rearrange("(a p) n -> a p n", p=128)   # [4, 128, DIM]
    out_t = out_d.rearrange("(a p) n -> a p n", p=128)      # [16, 128, DIM]

    SWAP_MASK = [(j + 1 if j % 2 == 0 else j - 1) for j in range(32)]

    with TileContext(nc) as tc, ExitStack() as top:
        # ---------------- persistent pools ----------------
        consts = top.enter_context(tc.tile_pool(name="consts", bufs=1))
        bq_s = consts.tile([128, NPAIR], F32)
        nc.gpsimd.dma_start(out=bq_s, in_=d["bqp"][:, :])
        bk_s = consts.tile([128, NPAIR], F32)
        nc.gpsimd.dma_start(out=bk_s, in_=d["bkp"][:, :])
        bv_s = consts.tile([1, DL], BF16)
        nc.gpsimd.dma_start(out=bv_s, in_=d["bv"][:, :])
        ones_s = consts.tile([1, 128], BF16)
        nc.gpsimd.dma_start(out=ones_s, in_=d["ones1"][:, :])
        cc_s = consts.tile([128, QL], BF16)
        nc.gpsimd.dma_start(out=cc_s, in_=d["cc"][:, :])
        ss_s = consts.tile([128, QL], BF16)
        nc.gpsimd.dma_start(out=ss_s, in_=d["ss"][:, :])
        warm = consts.tile([1, 8], F32)
        nc.scalar.activation(out=warm, in_=ones_s[0:1, 0:8], func=AFT.Exp)

        wq_pool = top.enter_context(tc.tile_pool(name="wq", bufs=1))
        wqs = [wq_pool.tile([128, DL], BF16, tag=f"wq{a}", name=f"wq{a}")
               for a in range(8)]
        kh_pool = top.enter_context(tc.tile_pool(name="kh", bufs=NPAIR))
        khT = [kh_pool.tile([128, KL], BF16, tag="kh", name=f"kh{p}")
               for p in range(NPAIR)]
        qh_pool = top.enter_context(tc.tile_pool(name="qh", bufs=2))
        vh_pool = top.enter_context(tc.tile_pool(name="vh", bufs=16))
        vh = [vh_pool.tile([128, NPAIR * 130], BF16, tag="vh", name=f"vh{t}")
              for t in range(16)]
        at_pool = top.enter_context(tc.tile_pool(name="atn", bufs=NPAIR))
        apT = [at_pool.tile([128, QL], BF16, tag="at", name=f"apT{p}")
               for p in range(NPAIR)]
        # q staging: two quarters [8][128,512] alive at once
        qst_pool = top.enter_context(tc.tile_pool(name="qst", bufs=1))
        qst = [[qst_pool.tile([128, 512], BF16, tag=f"qst{j}_{a}",
                              name=f"qs{j}_{a}")
                for a in range(8)] for j in range(2)]
        exp_pool = top.enter_context(tc.tile_pool(name="expp", bufs=EXP_BUFS))
        rot_pool = top.enter_context(tc.tile_pool(name="rot", bufs=1))
        uat_pool = top.enter_context(tc.tile_pool(name="uat", bufs=2))
        rc_pool = top.enter_context(tc.tile_pool(name="rc", bufs=2))
        bt_pool = top.enter_context(tc.tile_pool(name="bt", bufs=1))
        dscr = top.enter_context(tc.tile_pool(name="dscr", bufs=4, space="DRAM"))

        ring_pool = top.enter_context(
            tc.tile_pool(name="ring", bufs=1, space="PSUM"))
        ring = ring_pool.tile([128, 6 * 512], F32)
        ab_pool = top.enter_context(
            tc.tile_pool(name="ab", bufs=2, space="PSUM"))

        qh_unit = [None] * NUNIT

        # ---------------- emission state ----------------
        st = {"gs": 0}
        exp_tiles = {}
        pending = deque()     # (gate_gs, closure) strict FIFO with gates

        def pump(max_ops=4):
            n = 0
            while pending and n < max_ops and pending[0][0] <= st["gs"]:
                pending.popleft()[1]()
                n += 1

        def drain(gate_limit=10**9):
            while pending and pending[0][0] <= gate_limit:
                pending.popleft()[1]()

        def emit_score_chunk(u, mt, h, n):
            p, c2 = UNITS[u]
            gs = st["gs"]
            slot = gs % 6
            nc.tensor.matmul(
                ring[:, slot * 512:(slot + 1) * 512],
                lhsT=khT[p][h * 64:(h + 1) * 64, mt * 128:(mt + 1) * 128],
                rhs=qh_unit[u][h * 64:(h + 1) * 64, n * 512:(n + 1) * 512],
                start=True, stop=True,
                tile_position=(h * 64, 0),
            )
            if gs % 3 == 2:
                i = gs // 3
                et = exp_pool.tile([128, 1536], BF16, tag="exp")
                base = (slot - 2) * 512
                nc.scalar.activation(out=et, in_=ring[:, base:base + 1536],
                                     func=AFT.Exp, scale=0.125)
                exp_tiles[i] = et
            st["gs"] = gs + 1

        def exp_slice(c):
            i, off = c // 3, (c % 3) * 512
            return exp_tiles[i][:, off:off + 512]

        # ---------------- op builders ----------------
        def add_attn_group(u, h, n, gate_fn):
            """16 accumulating attn@V MMs + normalize for group (u,h,n)."""
            p, c2 = UNITS[u]
            base_c = u * 64
            pa = ab_pool.tile([128, 512], F32, tag="ab", name=f"pa{u}{h}{n}")

            def mk(t):
                def op():
                    nc.tensor.matmul(
                        pa[0:65, :],
                        lhsT=vh[t][:, p * 130 + h * 65: p * 130 + (h + 1) * 65],
                        rhs=exp_slice(base_c + t * 4 + n * 2 + h),
                        start=(t == 0), stop=(t == 15),
                    )
                return op
            last_gate = 0
            for t in range(16):
                last_gate = gate_fn(t)
                pending.append((last_gate, mk(t)))

            def norm():
                # evacuate PSUM immediately (frees the bank), then the
                # reciprocal runs out of SBUF via a [128,4] DRAM pack --
                # DVE reciprocal costs ~6.5 cyc/elem/partition, so the
                # single-partition form is ~17x slower.
                uat = uat_pool.tile([65, 512], BF16, tag="uat")
                nc.vector.tensor_copy(out=uat, in_=pa[0:65, :])
                ds = dscr.tile([1, 512], BF16, tag="ds")
                nc.sync.dma_start(out=ds, in_=uat[64:65, :])
                rc = rc_pool.tile([128, 4], BF16, tag="rc")
                nc.sync.dma_start(out=rc, in_=ds.rearrange("a (p e) -> (a p) e", p=128))
                with nc.allow_low_precision(reason="1/denominator in bf16: 0.2% rms, within 2e-2 budget"):
                    nc.vector.reciprocal(out=rc, in_=rc)
                ds2 = dscr.tile([1, 512], BF16, tag="ds2")
                nc.sync.dma_start(out=ds2.rearrange("a (p e) -> (a p) e", p=128), in_=rc)
                bt = bt_pool.tile([64, 512], BF16, tag="bt")
                nc.sync.dma_start(out=bt, in_=ds2[0:1, :].to_broadcast([64, 512]))
                qbase = c2 * 1024 + n * 512
                nc.vector.tensor_tensor(
                    out=apT[p][h * 64:(h + 1) * 64, qbase:qbase + 512],
                    in0=uat[0:64, :], in1=bt, op=ALU.mult)
            pending.append((last_gate, norm))

        def add_attn_half(u, n, base_gate, step=1):
            """Both heads' accumulation groups for q-half n of unit u."""
            for h in range(2):
                add_attn_group(
                    u, h, n,
                    lambda t, h=h: max(u * 64 + t * 4 + n * 2 + h + GATE,
                                       base_gate + t * step))

        def add_proj_chunk(ws, xs, b_s, dst, pcol, dslice, xslice, gate=0):
            """dst[:, dslice] = sum_a ws[a][:, pcol*128:+128].T @ xs[a] + b."""
            ps = ab_pool.tile([128, 512], F32, tag="ab")

            def mk(a):
                def op():
                    nc.tensor.matmul(
                        ps,
                        lhsT=ws[a][:, pcol * 128:(pcol + 1) * 128],
                        rhs=xs[a][:, xslice] if xslice is not None else xs[a][:, :],
                        start=(a == 0), stop=(a == 7),
                    )
                return op
            for a in range(8):
                pending.append((gate, mk(a)))

            def bias():
                nc.vector.tensor_scalar_add(
                    out=dst[:, dslice], in0=ps,
                    scalar1=b_s[:, pcol:pcol + 1])
            pending.append((gate, bias))

        def add_rotary(dst, dst_base, cs_base, gate=0):
            """Partial rotary on dst[:, dst_base:dst_base+512] in place."""
            def op():
                dcol = slice(dst_base, dst_base + 512)
                ccol = slice(cs_base, cs_base + 512)
                sw = rot_pool.tile([128, 512], BF16, tag="sw")
                nc.vector.stream_shuffle(out=sw, in_=dst[:, dcol],
                                         mask=SWAP_MASK)
                t1 = rot_pool.tile([128, 512], BF16, tag="t1")
                nc.vector.tensor_tensor(out=t1, in0=sw, in1=ss_s[:, ccol],
                                        op=ALU.mult)
                t2 = rot_pool.tile([128, 512], BF16, tag="t2")
                nc.vector.tensor_tensor(out=t2, in0=dst[:, dcol],
                                        in1=cc_s[:, ccol], op=ALU.mult)
                nc.vector.tensor_tensor(out=dst[:, dcol], in0=t1, in1=t2,
                                        op=ALU.add)
            pending.append((gate, op))

        def add_qproj(u, gate=0):
            p, c2 = UNITS[u]
            qh = qh_pool.tile([128, 1024], BF16, tag="qh", name=f"qh{u}")
            qh_unit[u] = qh
            for n in range(2):
                add_proj_chunk(wqs, qst[n], bq_s, qh, p,
                               slice(n * 512, (n + 1) * 512), None, gate=gate)
                add_rotary(qh, n * 512, c2 * 1024 + n * 512, gate=gate)

        # ---------------- scoped staging pools ----------------
        scA = ExitStack()
        wk_pool = scA.enter_context(tc.tile_pool(name="wk", bufs=1))
        wks = [wk_pool.tile([128, DL], BF16, tag=f"wk{a}", name=f"wk{a}")
               for a in range(8)]
        wv_pool = scA.enter_context(tc.tile_pool(name="wv", bufs=1))
        wvs = [wv_pool.tile([128, DL], BF16, tag=f"wv{a}", name=f"wv{a}")
               for a in range(8)]
        kv_pool = scA.enter_context(tc.tile_pool(name="kv", bufs=1))
        # 4 rotating quarter-sets: hold kT cols [s*512:(s+1)*512] for all 8
        # a-tiles, overwritten with the same vT quarter once kproj is done.
        kvq = [[kv_pool.tile([128, 512], BF16, tag=f"kvq{s}{a}",
                             name=f"kvq{s}{a}") for a in range(8)]
               for s in range(4)]

        def add_kproj_q(p, s, gate=0):
            """khT[p] columns of quarter s + rotary."""
            col = slice(s * 512, (s + 1) * 512)
            add_proj_chunk(wks, kvq[s], bk_s, khT[p], p, col, None, gate=gate)
            add_rotary(khT[p], s * 512, s * 512, gate=gate)

        def add_vdma(s, gate=0):
            def op():
                for a in range(8):
                    eng = nc.sync if a % 2 == 0 else nc.gpsimd
                    eng.dma_start(out=kvq[s][a],
                                  in_=vT_t[a][:, s * 512:(s + 1) * 512])
            pending.append((gate, op))

        def add_vproj(t, gate=0):
            s, r = divmod(t, 4)
            ps = ab_pool.tile([128, DL], F32, tag="ab")

            def mk(a):
                def op():
                    nc.tensor.matmul(
                        ps,
                        lhsT=kvq[s][a][:, r * 128:(r + 1) * 128],
                        rhs=wvs[a],
                        start=(a == 0), stop=False,
                    )
                return op
            for a in range(8):
                pending.append((gate, mk(a)))

            def tail():
                nc.tensor.matmul(ps, lhsT=ones_s, rhs=bv_s,
                                 start=False, stop=True)
                vtr = vh[t].rearrange("p (g h e) -> p g h e", h=2, e=65)
                nc.vector.memset(vtr[:, :, :, 64:65], 1.0)
                psr = ps.rearrange("p (g h e) -> p g h e", h=2, e=64)
                nc.vector.tensor_copy(out=vtr[:, :, :, 0:64], in_=psr)
            pending.append((gate, tail))

        # ---------------- staging DMAs ----------------
        for s in range(4):
            for a in range(8):
                eng = nc.sync if a % 2 == 0 else nc.gpsimd
                eng.dma_start(out=kvq[s][a],
                              in_=kT_t[a][:, s * 512:(s + 1) * 512])
        for a in range(8):
            nc.gpsimd.dma_start(out=wks[a], in_=wkT_t[a])
        for a in range(8):
            nc.gpsimd.dma_start(out=wqs[a], in_=wqT_t[a])
        for a in range(8):
            nc.sync.dma_start(out=qst[0][a], in_=qT_t[a][:, 0:512])
            nc.sync.dma_start(out=qst[1][a], in_=qT_t[a][:, 512:1024])
        for a in range(8):
            nc.gpsimd.dma_start(out=wvs[a], in_=wvT_t[a])

        # ---------------- head: kproj(p0,q0) + qproj(u0) ----------------
        add_kproj_q(0, 0)
        add_qproj(0)
        drain()

        wo_holder = {}
        ot_tiles = {}

        def add_outproj(qt, dc, pairs, add_to, gate):
            ps = ab_pool.tile([128, 512], F32, tag="ab")

            def mk(j, p):
                def op():
                    nc.tensor.matmul(
                        ps,
                        lhsT=apT[p][:, qt * 128:(qt + 1) * 128],
                        rhs=wo_holder["wo"][p][:, dc * 512:(dc + 1) * 512],
                        start=(j == 0), stop=(j == len(pairs) - 1),
                    )
                return op
            for j, p in enumerate(pairs):
                pending.append((gate, mk(j, p)))

            def out():
                ot = ot_tiles[qt]
                sl = slice(dc * 512, (dc + 1) * 512)
                if add_to:
                    nc.vector.tensor_tensor(out=ot[:, sl], in0=ps,
                                            in1=ot[:, sl], op=ALU.add)
                else:
                    nc.vector.tensor_copy(out=ot[:, sl], in_=ps)
            pending.append((gate, out))

        def add_out_dma(qt, gate):
            def op():
                nc.sync.dma_start(out=out_t[qt], in_=ot_tiles[qt])
            pending.append((gate, op))

        # ---------------- unit windows ----------------
        def add_attn_unit(u):
            w = u * 64
            # n0: lag-consumed through window u; n1: early next window
            add_attn_half(u, 0, w + 10, step=2)
            add_attn_half(u, 1, w + 66)

        for u in range(NUNIT):
            w = u * 64
            if u == 0:
                # quarter pipeline: finish kproj per quarter, swap in vT,
                # vproj, then unit-0 attnV; all inside window 0.
                for s in range(4):
                    for p in range(4):
                        if (p, s) != (0, 0):
                            add_kproj_q(p, s, gate=s * 6 + 2)
                    add_vdma(s, gate=s * 6 + 3)
                for t in range(16):
                    add_vproj(t, gate=8 + 2 * t)
                add_qproj(1, gate=34)
                add_attn_unit(0)
            else:
                if u < 7:
                    add_qproj(u + 1, gate=w + 2)
                if u == 2:
                    def reload_q():
                        for a in range(8):
                            nc.sync.dma_start(out=qst[0][a],
                                              in_=qT_t[a][:, 1024:1536])
                            nc.sync.dma_start(out=qst[1][a],
                                              in_=qT_t[a][:, 1536:2048])
                    pending.append((w + 12, reload_q))
                qt_map = {4: [0, 1, 2], 5: [3, 4, 5], 6: [6, 7]}
                if u in qt_map:
                    for qt in qt_map[u]:
                        for dc in range(2):
                            add_outproj(qt, dc, list(range(NPAIR)), False,
                                        gate=w + 6)
                        add_out_dma(qt, gate=w + 30)
                add_attn_unit(u)

            for mt in range(NMT):
                for n in range(2):
                    for h in range(2):
                        pump(6)
                        emit_score_chunk(u, mt, h, n)

            if u == 0:
                # staging consumers (kproj/vproj/vdma) all have gates
                # <= ~40; pop them so the staging pools can be released.
                drain(w + 40)
                scA.close()
                wo_pool = top.enter_context(tc.tile_pool(name="wo", bufs=1))
                wo_s = [wo_pool.tile([128, DIM], BF16, tag=f"wo{p}",
                                     name=f"wo{p}") for p in range(NPAIR)]
                for p in range(NPAIR):
                    nc.gpsimd.dma_start(out=wo_s[p], in_=woT_t[p])
                wo_holder["wo"] = wo_s
                ot_pool = top.enter_context(tc.tile_pool(name="ot", bufs=2))
                for qt in range(16):
                    ot_tiles[qt] = ot_pool.tile([128, DIM], F32, tag="ot",
                                                name=f"ot{qt}")

        # ---------------- tail ----------------
        # leftover exp: 512 chunks = 170 full N=1536 instrs + 2 chunks.
        # Dedicated tile (not the rotating pool) so its slot has no WAR
        # against unit 7's still-pending attn consumers.
        et = consts.tile([128, 1024], BF16)
        nc.scalar.activation(out=et, in_=ring[:, 0:1024],
                             func=AFT.Exp, scale=0.125)
        exp_tiles[170] = et
        drain()
        for qt in range(8, 16):
            for dc in range(2):
                add_outproj(qt, dc, list(range(NPAIR)), False, gate=0)
            add_out_dma(qt, gate=0)
        drain()

    nc.compile()
    _NC_CACHE["nc"] = nc
    return nc


def _make_in_maps(q, k, v, Wq, bq, Wk, bk, Wv, bv, Wo, bo):
    q, k, v = (np.asarray(x, np.float32) for x in (q, k, v))
    Wq, Wk, Wv, Wo = (np.asarray(x, np.float32) for x in (Wq, Wk, Wv, Wo))
    bq, bk, bv, bo = (np.asarray(x, np.float32) for x in (bq, bk, bv, bo))
    cc, ss = _rot_patterns()
    ones1 = np.ones((1, 128), np.float32)
    in_maps = []
    for c in range(NCORE):
        b, g = divmod(c, G)
        gs = slice(g * DL, (g + 1) * DL)
        in_maps.append({
            "qT": np.ascontiguousarray(q[b].T).astype(bf16),
            "kT": np.ascontiguousarray(k[b].T).astype(bf16),
            "vT": np.ascontiguousarray(v[b].T).astype(bf16),
            "wqT": np.ascontiguousarray(Wq[gs, :].T).astype(bf16),
            "wkT": np.ascontiguousarray(Wk[gs, :].T).astype(bf16),
            "wvT": np.ascontiguousarray(Wv[gs, :].T).astype(bf16),
            "woT": np.ascontiguousarray(Wo[:, gs].T).astype(bf16),
            "bqp": np.ascontiguousarray(bq[gs].reshape(NPAIR, 128).T),
            "bkp": np.ascontiguousarray(bk[gs].reshape(NPAIR, 128).T),
            "bv": np.ascontiguousarray(bv[gs][None, :]).astype(bf16),
            "ones1": ones1.astype(bf16),
            "cc": cc.astype(bf16), "ss": ss.astype(bf16),
        })
    return in_maps


def run(inputs: dict, trace: bool = False, tmpdir: str | None = None):
    """Returns (out [B, QL, DIM] f32, exec_time_ns or None)."""
    from concourse.bass_utils import run_bass_kernel_spmd

    nc = _build_nc()
    in_maps = _make_in_maps(**inputs)
    res = run_bass_kernel_spmd(nc, in_maps, list(range(NCORE)), trace=trace,
                               tmpdir=tmpdir)
    bo = np.asarray(inputs["bo"], np.float32)
    outs = [res.results[i]["out"] for i in range(NCORE)]
    out = np.stack([outs[G * b] + outs[G * b + 1] for b in range(B)])
    out += bo[None, None, :]
    return out.astype(np.float32), res.exec_time_ns


def kernel(**inputs) -> np.ndarray:
    out, _ = run(inputs, trace=False)
    return out
